# revision 1
# baseline (speedup 1.0000x reference)
"""NodeAttAggregator Trainium2 kernel (v3: fp16 node gather design).

Math (reference):
    q = hedge @ Wq;  k = node @ Wq
    S = (q @ k.T) / sqrt(FOUT)  masked to member (row,col) pairs, row-softmax
    out = (softmax(S) @ k).T                       # [FOUT, H]

Identities used (k rows never materialized):
    S[r, c]  = (hedge_s[r] @ (Wq Wq.T)) . node[c]      (hedge_s pre-scaled)
    out[:,r] = Wq.T @ (sum_c attn[r,c] * node[c])

Per core (512 hyperedge rows, 4 blocks of 128 rows on partitions):
  - gather the 128x32 member node rows (fp16, 512B each) with per-slot
    indirect DMAs on the GpSimd queue — the serial dma stream is the
    kernel's wall; every compute stage hides underneath it.
  - scores s[r,j] = qG16[r] . ng16[r,j] via DVE scalar_tensor_tensor
    with accum_out (qG = hedge_s @ Wq Wq.T, computed on PE in setup).
  - softmax over the 32 member slots (dup pairs suppressed by a host
    -1e30 bias), normalized weights En in fp16.
  - aggregation aggN[r] = sum_j En[r,j] ng[r,j] as 32 PSUM-accumulated
    matmuls  aggN += diag(En_j) @ ng_j ; the diagonal weight matrices
    (e-major [p, e, j] layout) are rebuilt per block as ident*En_j by
    copy-with-scale ops split across ACT and DVE (a DVE scalar-chain
    fallback exists under AGG_MODE="chain").
  - output block out.T = Wq.T @ aggN.T via PE transposes + matmuls.
Host ships fp16 node/hedgeT/Wq and wrapped int32 gather indices; the
host transposes the assembled [H, FOUT] result.
"""

import numpy as np

H, N, FIN, FOUT, DEG = 4096, 20000, 256, 128, 32
NCORES = 8
RPC = H // NCORES            # 512 rows per core
NBLK = RPC // 128            # 4 blocks of 128 rows
SCALE = 1.0 / float(np.sqrt(np.float32(FOUT)))
AGG_MODE = "diag"            # "diag" (PE) | "chain" (DVE/ACT fallback)

_CACHE = {}


def _build_nc(agg_mode=None):
    import concourse.bacc as bacc
    import concourse.bass as bass
    import concourse.mybir as mybir
    import bass_rust
    from concourse.tile import TileContext

    if agg_mode is None:
        agg_mode = AGG_MODE
    f32 = mybir.dt.float32
    f16 = mybir.dt.float16
    i32 = mybir.dt.int32
    Alu = mybir.AluOpType
    Act = mybir.ActivationFunctionType

    nc = bacc.Bacc()
    node16 = nc.declare_dram_parameter("node16", [N, FIN], f16, isOutput=False)
    hedgeT = nc.declare_dram_parameter("hedgeT16", [FIN, RPC], f16, isOutput=False)
    wq_in = nc.declare_dram_parameter("wq16", [FIN, FOUT], f16, isOutput=False)
    idx_in = nc.declare_dram_parameter("idx32", [128, NBLK * DEG], i32, isOutput=False)
    bias_in = nc.declare_dram_parameter("bias_t", [128, NBLK * DEG], f32, isOutput=False)
    out_d = nc.declare_dram_parameter("outT", [FOUT, RPC], f32, isOutput=True)

    with TileContext(nc) as tc:
        with (
            tc.tile_pool(name="const", bufs=1) as constp,
            tc.tile_pool(name="ng", bufs=1) as ngp,
            tc.tile_pool(name="work", bufs=2) as workp,
            tc.tile_pool(name="ps", bufs=2, space="PSUM") as psp,
            tc.tile_pool(name="psag", bufs=2, space="PSUM") as psagp,
        ):
            # ---- setup (idx load first: the gather stream gates on it)
            idxt = constp.tile([128, NBLK * DEG], i32)
            nc.gpsimd.dma_start(idxt[:], idx_in[:])
            wq16 = constp.tile([128, 2, FOUT], f16)
            nc.sync.dma_start(
                wq16[:], wq_in[:].rearrange("(a p) f -> p a f", p=128)
            )
            hT = constp.tile([128, 2, RPC], f16)
            nc.scalar.dma_start(
                hT[:], hedgeT[:].rearrange("(a p) r -> p a r", p=128)
            )
            biast = constp.tile([128, NBLK * DEG], f32)
            nc.scalar.dma_start(biast[:], bias_in[:])

            ident = constp.tile([128, 128], f16)
            from concourse import masks
            masks.make_identity(nc, ident[:])

            # ---- gathers first: the serial Pool DMA stream is the wall
            ngs = []
            for b in range(NBLK):
                ng = ngp.tile([128, DEG, FIN], f16, tag=f"ng{b}")
                ng3 = ng[:]
                for j in range(DEG):
                    col = b * DEG + j
                    nc.gpsimd.indirect_dma_start(
                        out=ng3[:, j, :],
                        out_offset=None,
                        in_=node16[:],
                        in_offset=bass.IndirectOffsetOnAxis(
                            ap=idxt[:, col : col + 1], axis=0
                        ),
                    )
                ngs.append(ng)

            # WqT [f, fin] via PE transposes (for the output projection)
            wqT = constp.tile([128, FIN], f16)
            for a in range(2):
                ps = psp.tile([128, 128], f16, tag="tr")
                nc.tensor.transpose(ps[:], wq16[:, a, :], ident[:])
                nc.scalar.activation(
                    wqT[:, a * 128 : (a + 1) * 128], ps[:], func=Act.Copy
                )
            # G = Wq @ Wq.T [256, 256] fp16, chunked [fin_a, 2, 256]
            g16 = constp.tile([128, 2, FIN], f16)
            for a in range(2):
                ps = psp.tile([128, FIN], f32, tag="mmg")
                nc.tensor.matmul(
                    ps[:], wqT[:, a * 128 : (a + 1) * 128], wqT[:],
                    start=True, stop=True,
                )
                nc.scalar.activation(g16[:, a, :], ps[:], func=Act.Copy)
            # qG[r, fin] per block = hedge_s @ G  (scores operand)
            qG = constp.tile([128, NBLK, FIN], f16)
            for b in range(NBLK):
                ps = psp.tile([128, FIN], f32, tag="mmg")
                for a in range(2):
                    nc.tensor.matmul(
                        ps[:], hT[:, a, b * 128 : (b + 1) * 128], g16[:, a, :],
                        start=(a == 0), stop=(a == 1),
                    )
                nc.vector.tensor_copy(qG[:, b, :], ps[:])

            # diagonal-weight buffers (e-major [p, e, j]; the diagonal DMA
            # writes 32-element contiguous runs), zeroed once
            walls = []
            if agg_mode == "diag":
                for w in range(2):
                    wl = constp.tile([128, 128, DEG], f16, tag=f"wl{w}")
                    walls.append(wl)

            S = constp.tile([128, NBLK * DEG], f32)
            Sm = constp.tile([128, NBLK * DEG], f32)
            E = constp.tile([128, NBLK * DEG], f32)
            Z = constp.tile([128, NBLK], f32)
            Zi = constp.tile([128, NBLK], f32)
            junkd = constp.tile([128, FIN], f16)

            N_WARM = 8
            for b in range(NBLK):
                ng3 = ngs[b][:]
                if b == NBLK - 1:
                    # tail p-state warm-up: junk matmuls triggered by this
                    # block's gather j=27 (~3us before the aggs are ready)
                    # keep PE continuously busy through the idle window so
                    # the final agg burst starts at peak clock
                    wps = psp.tile([128, FIN], f32, tag="mmg")
                    for wmm in range(N_WARM):
                        nc.tensor.matmul(
                            wps[:], wq16[:, 0, :], ng3[:, 25, :],
                            start=True, stop=True,
                        )
                for j in range(DEG):
                    col = b * DEG + j
                    nc.vector.scalar_tensor_tensor(
                        out=junkd[:],
                        in0=ng3[:, j, :],
                        scalar=0.0,
                        in1=qG[:, b, :],
                        op0=Alu.bypass,
                        op1=Alu.mult,
                        accum_out=S[:, col : col + 1],
                    )

                bs = slice(b * DEG, (b + 1) * DEG)
                nc.vector.tensor_tensor(
                    out=Sm[:, bs], in0=S[:, bs], in1=biast[:, bs], op=Alu.add
                )
                nc.scalar.activation(
                    out=E[:, bs], in_=Sm[:, bs], func=Act.Exp,
                    accum_out=Z[:, b : b + 1],
                )
                nc.vector.reciprocal(Zi[:, b : b + 1], Z[:, b : b + 1])
                enf = workp.tile([128, DEG], f32, tag="en")
                nc.vector.tensor_scalar(
                    out=enf[:], in0=E[:, bs],
                    scalar1=Zi[:, b : b + 1], scalar2=None, op0=Alu.mult,
                )

                # aggN[r, fin] = sum_j En[r, j] * ng[r, j, :]
                aggps = psagp.tile([128, FIN], f32, tag="agg")
                if agg_mode == "diag":
                    wall = walls[b % 2]
                    # wall[:, :, j] = ident * En[:, j] (full rewrite, no
                    # stale state; standard strided engine writes)
                    for j in range(DEG):
                        col = b * DEG + j
                        weng = nc.vector if j < 22 else nc.scalar
                        if weng is nc.scalar:
                            weng.activation(
                                wall[:, :, j], ident[:], func=Act.Copy,
                                scale=enf[:, j : j + 1],
                            )
                        else:
                            weng.tensor_scalar(
                                out=wall[:, :, j], in0=ident[:],
                                scalar1=enf[:, j : j + 1], scalar2=None,
                                op0=Alu.mult,
                            )
                    for j in range(DEG):
                        nc.tensor.matmul(
                            aggps[:], wall[:, :, j], ng3[:, j, :],
                            start=(j == 0), stop=(j == DEG - 1),
                        )
                    aggsbA = workp.tile([128, 128], f16, tag="agsbA")
                    aggsbB = workp.tile([128, 128], f16, tag="agsbB")
                    nc.scalar.activation(
                        aggsbA[:], aggps[:, 0:128], func=Act.Copy
                    )
                    nc.vector.tensor_copy(aggsbB[:], aggps[:, 128:256])
                else:
                    # fallback: DVE scalar chain (baseline-proven ops)
                    accA = workp.tile([128, FIN], f16, tag="accA")
                    accB = workp.tile([128, FIN], f16, tag="accB")
                    nc.vector.tensor_scalar(
                        out=accA[:], in0=ng3[:, 0, :],
                        scalar1=E[:, b * DEG : b * DEG + 1], scalar2=None,
                        op0=Alu.mult,
                    )
                    cur, nxt = accA, accB
                    for j in range(1, DEG):
                        col = b * DEG + j
                        nc.vector.scalar_tensor_tensor(
                            out=nxt[:], in0=ng3[:, j, :],
                            scalar=E[:, col : col + 1], in1=cur[:],
                            op0=Alu.mult, op1=Alu.add,
                        )
                        cur, nxt = nxt, cur
                    aggsb = workp.tile([128, FIN], f16, tag="aggsb")
                    nc.vector.tensor_scalar(
                        out=aggsb[:], in0=cur[:],
                        scalar1=Zi[:, b : b + 1], scalar2=None, op0=Alu.mult,
                    )
                    aggsbA = aggsb[:, 0:128]
                    aggsbB = aggsb[:, 128:256]

                # out.T block = Wq.T @ aggN.T : transpose aggN then matmul
                # (A/B halves evac on different engines so they overlap)
                aA = aggsbA[:] if agg_mode == "diag" else aggsbA
                aB = aggsbB[:] if agg_mode == "diag" else aggsbB
                atps0 = psp.tile([128, 128], f16, tag="tr")
                nc.tensor.transpose(atps0[:], aA, ident[:])
                at16a = workp.tile([128, 128], f16, tag="ata")
                nc.vector.tensor_copy(at16a[:], atps0[:])
                atps1 = psp.tile([128, 128], f16, tag="tr")
                nc.tensor.transpose(atps1[:], aB, ident[:])
                at16b = workp.tile([128, 128], f16, tag="atb")
                nc.vector.tensor_copy(at16b[:], atps1[:])

                ops = psagp.tile([128, 128], f32, tag="out")
                nc.tensor.matmul(
                    ops[:], wq16[:, 0, :], at16a[:], start=True, stop=False
                )
                nc.tensor.matmul(
                    ops[:], wq16[:, 1, :], at16b[:], start=False, stop=True
                )
                osb = workp.tile([128, 128], f32, tag="osb")
                nc.vector.tensor_copy(osb[:], ops[:])
                nc.sync.dma_start(
                    out_d[:, b * 128 : (b + 1) * 128], osb[:]
                )

    nc.finalize()
    return nc


def get_nc():
    key = ("nc", AGG_MODE)
    if key not in _CACHE:
        _CACHE[key] = _build_nc(AGG_MODE)
    return _CACHE[key]


def make_in_maps(hedge_embed, node_embed, Wq, row_idx, col_idx):
    """Host-side sharding: fp16 operands, per-core indices, dup bias."""
    hedge_embed = np.asarray(hedge_embed, dtype=np.float32)
    node_embed = np.asarray(node_embed, dtype=np.float32)
    Wq = np.asarray(Wq, dtype=np.float32)
    row_idx = np.asarray(row_idx).astype(np.int64)
    col_idx = np.asarray(col_idx).astype(np.int64)

    expect = np.repeat(np.arange(H, dtype=np.int64), DEG)
    if np.array_equal(row_idx, expect):
        cols = col_idx.reshape(H, DEG)
    else:
        order = np.argsort(row_idx, kind="stable")
        assert np.array_equal(row_idx[order], expect), "rows must have DEG pairs"
        cols = col_idx[order].reshape(H, DEG)

    # duplicate (row,col) pairs beyond the first get -1e30 score bias
    order = np.argsort(cols, axis=1, kind="stable")
    sc = np.take_along_axis(cols, order, axis=1)
    dup_sorted = np.zeros_like(sc, dtype=bool)
    dup_sorted[:, 1:] = sc[:, 1:] == sc[:, :-1]
    dup = np.zeros((H, DEG), dtype=bool)
    np.put_along_axis(dup, order, dup_sorted, axis=1)
    bias = np.where(dup, np.float32(-1e30), np.float32(0.0))

    node16 = node_embed.astype(np.float16)
    hedgeT16 = (hedge_embed * np.float32(SCALE)).T.astype(np.float16)
    wq16 = Wq.astype(np.float16)

    in_maps = []
    for c in range(NCORES):
        r0 = c * RPC
        ccols = cols[r0 : r0 + RPC]
        idx32 = np.empty((128, NBLK * DEG), np.int32)
        bias_t = np.empty((128, NBLK * DEG), np.float32)
        for b in range(NBLK):
            blk = ccols[b * 128 : (b + 1) * 128]          # [128 p, 32 j]
            idx32[:, b * DEG : (b + 1) * DEG] = blk
            bias_t[:, b * DEG : (b + 1) * DEG] = bias[
                r0 + b * 128 : r0 + (b + 1) * 128
            ]
        in_maps.append({
            "node16": node16,
            "hedgeT16": np.ascontiguousarray(hedgeT16[:, r0 : r0 + RPC]),
            "wq16": wq16,
            "idx32": idx32,
            "bias_t": bias_t,
        })
    return in_maps


def run(in_maps, **kwargs):
    from concourse.bass_utils import run_bass_kernel_spmd

    nc = get_nc()
    return run_bass_kernel_spmd(nc, in_maps, list(range(NCORES)), **kwargs)


def kernel(hedge_embed, node_embed, Wq, row_idx, col_idx):
    in_maps = make_in_maps(hedge_embed, node_embed, Wq, row_idx, col_idx)
    res = run(in_maps)
    out = np.concatenate(
        [res.results[c]["outT"] for c in range(NCORES)], axis=1
    )
    return np.ascontiguousarray(out.astype(np.float32))

